# revision 54
# baseline (speedup 1.0000x reference)
"""Trainium2 Bass kernel for the vq_codebook / HDC problem.

Math (reference):
    hv      = sign(feat @ proj_w.T)                  [N=16384, D=10000], +-1 (0 -> +1)
    per_cls = segment_sum(hv, labels, K=3)           [3, D]
    updated = classify_weights + 0.5 * per_cls
    protos  = updated / max(||updated||_row, eps)
    logits  = hv @ protos.T                          [N, 3]

Strategy (8 NeuronCores, D-sharded, no collectives):
  * Each core owns DLOC=1250 hyper-dims (10 tiles of PT=125), all N rows.
    Host sorts rows by label so segment sums are contiguous-range sums.
  * mm1: z = projw_loc.T-tiles @ featT in fp32r (1 cyc/row), psum chunks
    [125, 1024].
  * sign: three engines in parallel convert z-psum into fp8 hv tiles:
      A-tiles (ACT):      Sign   -> hv in {-1, +1},  accum = S
      B-tiles (DVE/Pool): is_ge  -> hv in {0, 1},    accum = count = (S+len)/2
    (with accum_out, tensor_scalar's op1 is the reduce op, so one op can
    only produce {0,1}; the count->S shift folds into the host-side
    cwb = 2cw - len_k input for B tiles.)
  * mm2: fp8 DoubleRow matmuls (0.5 cyc/row, both d-tiles of a pair
    contracted per instr).  Stationary per pair: [125, 2, 12] fp8 packing
    hi/lo splits of u2 = 2*cw + S, with A-tiles in column block 0:6 and
    B-tiles in 6:12 -> psum rows 0:6 / 6:12.  A "ones" DoubleRow matmul
    gives the exact device-side column sums of the B stationaries.
    Host: P2 = (p0+p3) + 2*(p6+p9) - (cs6+cs9),
          logits = P2 / (2*||updated||).
  * Dummy matmuls keep the PE pstate ramped across the finalize window
    between mm1 and mm2; p_out is drained straight from PSUM by DMAs
    issued on the (otherwise idle) gpsimd queue.
"""

import os
import sys

sys.path.insert(0, "/opt/trn_rl_repo")
os.environ.setdefault("MYCRO_LOCAL_CACHE", "1")

import numpy as np

import concourse.bass as bass
import concourse.tile as tile
from concourse import bacc
from concourse import mybir
from concourse.bass import MemorySpace
from concourse.bass_utils import run_bass_kernel_spmd

# ---------------------------------------------------------------- constants
N = 16384          # rows
C = 128            # feat dim (contraction)
D = 10000          # hyper dim
K = 3              # classes
NCORES = 8
DLOC = D // NCORES          # 1250 per core
PT = 125                    # partitions per d-tile
NT = DLOC // PT             # 10 d-tiles per core
NPAIR = NT // 2             # 5 fp8 DoubleRow tile pairs
SCH = 2048                  # mm1 superchunk columns (4 psum banks)
NJ = N // SCH               # 8 superchunks
MCH = 512                   # mm1 matmul / mm2 output chunk columns
NC2 = N // MCH              # 32 mm2 chunks
MM_DT = mybir.dt.float32r   # encode-matmul dtype (1 cyc/row)
FP8 = mybir.dt.float8e4

# Only ACT and DVE can read PSUM (GPSIMD/Pool cannot), so the sign work
# has exactly two exit lanes: even tiles on ACT (Sign, +-1), odd tiles
# on DVE (is_ge, {0,1}).  With both lanes saturated the sign phase is
# the kernel's critical path (~2x the PE's mm1 time).
A_TILES = (0, 2, 4, 6, 8)   # ACT-signed tiles (+-1); odd tiles are {0,1}
N_DUMMY = 16                # PE keep-warm matmuls between mm1 and mm2

LAM = 0.5
EPS = 1e-12

LAST_RESULTS = None         # BassKernelResults of the most recent run (for test.py)


def _superchunk_segments(cuts):
    """Per superchunk j: list of (s0, s1, cls) relative to the superchunk,
    split at sorted-label class boundaries."""
    table = []
    for j in range(NJ):
        lo, hi = j * SCH, (j + 1) * SCH
        pts = [lo] + [b for b in cuts if lo < b < hi] + [hi]
        segs = []
        for a, b in zip(pts[:-1], pts[1:]):
            cls = 0 if a < cuts[0] else (1 if a < cuts[1] else 2)
            segs.append((a - lo, b - lo, cls))
        table.append(segs)
    return table


def _assign_engines():
    """Fixed two-lane assignment: A-tiles on ACT, B-tiles on DVE."""
    return [["A" if t in A_TILES else "D" for t in range(NT)] for j in range(NJ)]


def build_nc(cuts):
    """Build the single-core Bass program (same for all cores; only DRAM
    inputs differ per core).  cuts = [c0, c0+c1] sorted-label boundaries."""
    nc = bacc.Bacc()
    featT = nc.dram_tensor("featT", [C, N], MM_DT, kind="ExternalInput")
    projwT = nc.dram_tensor("projwT", [C, DLOC], MM_DT, kind="ExternalInput")
    cwb = nc.dram_tensor("cwb", [PT, NT * K], mybir.dt.float32, kind="ExternalInput")
    p_out = nc.dram_tensor("p_out", [4 * K, N], mybir.dt.float32, kind="ExternalOutput")
    cs_out = nc.dram_tensor("cs_out", [4 * K, 1], mybir.dt.float32, kind="ExternalOutput")
    s_out = nc.dram_tensor("s_out", [PT, NT * K], mybir.dt.float32, kind="ExternalOutput")

    seg_table = _superchunk_segments(cuts)
    ncols = sum(len(s) for s in seg_table)        # accum columns per tile
    col_cls = [cls for segs in seg_table for (_a, _b, cls) in segs]
    eng = _assign_engines()

    with tile.TileContext(nc) as tc:
        with (
            tc.tile_pool(name="singles", bufs=1) as singles,
            tc.tile_pool(name="feat", bufs=2) as featp,
            tc.tile_pool(name="u2f", bufs=2) as u2fp,
            tc.tile_pool(name="pstage", bufs=4) as pstp,
        ):
            # hv tiles first so their SBUF byte offsets stay 16B-aligned
            # (DoubleRow rhs requires 2B-aligned partition addresses)
            hv = [singles.tile([PT, 2, N], FP8, name=f"hv{p}") for p in range(NPAIR)]
            projw_sb = singles.tile([C, DLOC], MM_DT)
            # all DMA issues ride the gpsimd SWDGE queue: walrus reassigns
            # hwdge (SP) DMAs onto the ACT sequencer, which must stay free
            # for sign work
            nc.gpsimd.dma_start(out=projw_sb[:, :5 * PT], in_=projwT[:, :5 * PT])
            cwb_sb = singles.tile([PT, NT * K], mybir.dt.float32)
            s_sb = singles.tile([PT, NT * K], mybir.dt.float32)
            spart = singles.tile([PT, NT * ncols], mybir.dt.float32)
            # DoubleRow lhsT outer free step must be 16B-aligned -> pad the
            # per-plane stationary stride from 12 to 16 columns
            stat = [singles.tile([PT, 2, 16], FP8, name=f"st{p}") for p in range(NPAIR)]
            mones = singles.tile([PT, 2, 16], FP8)
            dums = singles.tile([C, MCH], mybir.dt.bfloat16)
            for p in range(NPAIR):
                nc.vector.memset(stat[p], 0.0)
            nc.vector.memset(mones, 1.0)
            nc.vector.memset(dums, 0.0)

            # ---- produce: z psum chunks -> fp8 hv tiles + segment sums ----
            with tc.tile_pool(name="mm1ps", bufs=2, space=MemorySpace.PSUM) as mm1ps:
                for j in range(NJ):
                    fj = featp.tile([C, SCH], MM_DT, tag="fj")
                    if j == 0:
                        # half-DMAs so mm1(0,0) starts after 256 KB
                        nc.gpsimd.dma_start(
                            out=fj[:, :MCH], in_=featT[:, :MCH])
                        nc.gpsimd.dma_start(
                            out=fj[:, MCH:], in_=featT[:, MCH:SCH])
                        nc.gpsimd.dma_start(
                            out=projw_sb[:, 5 * PT:], in_=projwT[:, 5 * PT:]
                        )
                    else:
                        nc.gpsimd.dma_start(
                            out=fj, in_=featT[:, j * SCH:(j + 1) * SCH])
                    if j == 2:
                        nc.gpsimd.dma_start(out=cwb_sb, in_=cwb[:, :])
                    for t in range(NT):
                        ps = mm1ps.tile([PT, SCH], mybir.dt.float32, tag="mm1")
                        for h in range(SCH // MCH):
                            nc.tensor.matmul(
                                ps[:, h * MCH:(h + 1) * MCH],
                                projw_sb[:, t * PT:(t + 1) * PT],
                                fj[:, h * MCH:(h + 1) * MCH],
                                start=True, stop=True,
                            )
                        col0 = sum(len(seg_table[jj]) for jj in range(j))
                        e = eng[j][t]
                        for si, (s0, s1, _cls) in enumerate(seg_table[j]):
                            hv_sl = hv[t // 2][:, t % 2, j * SCH + s0: j * SCH + s1]
                            acc = spart[:, t * ncols + col0 + si: t * ncols + col0 + si + 1]
                            if e == "A":
                                nc.scalar.activation(
                                    hv_sl, ps[:, s0:s1],
                                    mybir.ActivationFunctionType.Sign,
                                    accum_out=acc,
                                )
                            else:
                                # {0,1} in one op; accum = count of positives
                                # (op1 is the accum reduce op, not elementwise)
                                nc.vector.tensor_scalar(
                                    hv_sl, ps[:, s0:s1], 0.0, None,
                                    mybir.AluOpType.is_ge, mybir.AluOpType.add,
                                    accum_out=acc,
                                )

            # ---- per-tile finalize: S, u2/2 = m*s + cwb, fp8 hi/lo split --
            # (fp8e4 is IEEE e4m3, max 240; u2 reaches ~310, so the
            #  stationaries hold u2/2 and the host doubles the blocks)
            # Both exit lanes are still draining the last signs here, so
            # A-tile reduces ride ACT's Copy+accum path, B-tile reduces use
            # DVE's reduce, and the fp8 quantization runs on the idle Pool.
            red_scratch = singles.tile([PT, 8], mybir.dt.float32)
            for t in range(NT):
                is_a = t in A_TILES
                m_t = 0.5 if is_a else 1.0
                for k in range(K):
                    idxs = [i for i, cc in enumerate(col_cls) if cc == k]
                    a, b = idxs[0], idxs[-1] + 1
                    s_col = s_sb[:, t * K + k: t * K + k + 1]
                    cols = spart[:, t * ncols + a: t * ncols + b]
                    if is_a:
                        nc.scalar.activation(
                            red_scratch[:, :b - a], cols,
                            mybir.ActivationFunctionType.Copy,
                            accum_out=s_col,
                        )
                    else:
                        nc.vector.reduce_sum(
                            s_col, cols, axis=mybir.AxisListType.X,
                        )
                u2f = u2fp.tile([PT, K], mybir.dt.float32, tag="u2f")
                nc.vector.scalar_tensor_tensor(
                    u2f, s_sb[:, t * K:(t + 1) * K], m_t,
                    cwb_sb[:, t * K:(t + 1) * K],
                    mybir.AluOpType.mult, mybir.AluOpType.add,
                )
                blk = 0 if is_a else 2 * K   # column block in stationary
                st = stat[t // 2]
                hi = st[:, t % 2, blk: blk + K]
                lo = st[:, t % 2, blk + K: blk + 2 * K]
                nc.scalar.activation(
                    hi, u2f, mybir.ActivationFunctionType.Copy
                )
                nc.vector.scalar_tensor_tensor(
                    lo, u2f, 1.0, hi,
                    mybir.AluOpType.mult, mybir.AluOpType.subtract,
                )

            nc.gpsimd.dma_start(out=s_out[:, :], in_=s_sb)

            with tc.tile_pool(name="pps", bufs=4, space=MemorySpace.PSUM) as pps:
                # ---- PE keep-warm bridge over the finalize window ---------
                for i in range(N_DUMMY):
                    dpp = pps.tile([128, MCH], mybir.dt.float32, tag="pp")
                    nc.tensor.matmul(
                        dpp[:PT, :], dums[:, 0:PT], dums,
                        start=True, stop=True,
                    )

                # ---- stationary column sums (exact {0,1} fixup) -----------
                ppc = pps.tile([128, MCH], mybir.dt.float32, tag="pp")
                for p in range(NPAIR):
                    nc.tensor.matmul(
                        ppc[:4 * K, 0:1], stat[p][:, :, 0:4 * K],
                        mones[:, :, 0:1],
                        start=(p == 0), stop=(p == NPAIR - 1),
                        perf_mode=mybir.MatmulPerfMode.DoubleRow,
                    )
                csst = pstp.tile([4 * K, 1], mybir.dt.float32, tag="csst")
                nc.vector.tensor_copy(csst, ppc[:4 * K, 0:1])
                nc.gpsimd.dma_start(out=cs_out[:, :], in_=csst)

                # ---- mm2: P2 partials via fp8 DoubleRow -------------------
                # drain copies alternate DVE/ACT; two chunks share one
                # staging tile; DMA issues alternate SP / gpsimd queues
                GRP = 2
                pst = None
                for c in range(NC2):
                    pp = pps.tile([128, MCH], mybir.dt.float32, tag="pp")
                    for p in range(NPAIR):
                        nc.tensor.matmul(
                            pp[:4 * K, :], stat[p][:, :, 0:4 * K],
                            hv[p][:, :, c * MCH:(c + 1) * MCH],
                            start=(p == 0), stop=(p == NPAIR - 1),
                            perf_mode=mybir.MatmulPerfMode.DoubleRow,
                        )
                    if c % GRP == 0:
                        pst = pstp.tile([4 * K, GRP * MCH], mybir.dt.float32, tag="pst")
                    half = pst[:, (c % GRP) * MCH:(c % GRP + 1) * MCH]
                    # only ACT/DVE can read PSUM
                    if c % 2 == 0:
                        nc.vector.tensor_copy(half, pp[:4 * K, :])
                    else:
                        nc.scalar.activation(
                            half, pp[:4 * K, :], mybir.ActivationFunctionType.Copy
                        )
                    if c % GRP == GRP - 1:
                        nc.gpsimd.dma_start(
                            out=p_out[:, (c - GRP + 1) * MCH:(c + 1) * MCH],
                            in_=pst,
                        )
    nc.compile()
    return nc


def _prep_inputs(feat_s, proj_w, classify_weights, counts):
    featT = np.ascontiguousarray(feat_s.T).astype(np.float32)  # [128, N]
    in_maps = []
    for core in range(NCORES):
        sl = slice(core * DLOC, (core + 1) * DLOC)
        projwT = np.ascontiguousarray(proj_w[sl].T).astype(np.float32)  # [128, DLOC]
        # stationaries hold u2/2 = cw + S/2; for {0,1} tiles S = 2c - len
        cw2 = classify_weights[:, sl].astype(np.float32).T              # [DLOC, 3]
        for t in range(NT):
            if t not in A_TILES:
                cw2[t * PT:(t + 1) * PT, :] -= 0.5 * counts[None, :].astype(np.float32)
        cwb = np.ascontiguousarray(
            cw2.reshape(NT, PT, K).transpose(1, 0, 2).reshape(PT, NT * K)
        )
        in_maps.append({"featT": featT, "projwT": projwT, "cwb": cwb})
    return in_maps


def kernel(feat, proj_w, classify_weights, labels, _trace=False):
    global LAST_RESULTS
    feat = np.asarray(feat, dtype=np.float32)
    proj_w = np.asarray(proj_w, dtype=np.float32)
    classify_weights = np.asarray(classify_weights, dtype=np.float32)
    labels = np.asarray(labels).astype(np.int64)

    perm = np.argsort(labels, kind="stable")
    feat_s = feat[perm]
    counts = np.bincount(labels, minlength=K)
    cuts = [int(counts[0]), int(counts[0] + counts[1])]

    nc = build_nc(cuts)
    in_maps = _prep_inputs(feat_s, proj_w, classify_weights, counts)
    res = run_bass_kernel_spmd(nc, in_maps, list(range(NCORES)), trace=_trace)
    LAST_RESULTS = res

    S = np.zeros((K, D), np.float32)
    P2 = np.zeros((K, N), np.float64)
    for core in range(NCORES):
        s_raw = np.asarray(res.results[core]["s_out"])        # [PT, NT*K]
        s_full = s_raw.reshape(PT, NT, K).transpose(1, 0, 2)  # [NT, PT, K]
        for t in range(NT):
            if t in A_TILES:
                st = s_full[t]
            else:
                st = 2.0 * s_full[t] - counts[None, :].astype(np.float32)
            S[:, core * DLOC + t * PT: core * DLOC + (t + 1) * PT] = st.T
        p = np.asarray(res.results[core]["p_out"]).astype(np.float64)   # [12, N]
        cs = np.asarray(res.results[core]["cs_out"]).astype(np.float64)  # [12, 1]
        P2 += 2.0 * ((p[0:3] + p[3:6]) + 2.0 * (p[6:9] + p[9:12])
                     - (cs[6:9] + cs[9:12]))

    updated = classify_weights + np.float32(LAM) * S          # [K, D] f32
    norms = np.linalg.norm(updated, axis=1)
    scale = 0.5 / np.maximum(norms, EPS)
    logits_sorted = (P2 * scale[:, None]).T.astype(np.float32)  # [N, K]
    out = np.empty((N, K), np.float32)
    out[perm] = logits_sorted
    return out


# revision 56
# speedup vs baseline: 1.5191x; 1.5191x over previous
"""Trainium2 Bass kernel for the vq_codebook / HDC problem.

Math (reference):
    hv      = sign(feat @ proj_w.T)                  [N=16384, D=10000], +-1 (0 -> +1)
    per_cls = segment_sum(hv, labels, K=3)           [3, D]
    updated = classify_weights + 0.5 * per_cls
    protos  = updated / max(||updated||_row, eps)
    logits  = hv @ protos.T                          [N, 3]

Strategy (8 NeuronCores, D-sharded, no collectives):
  * Each core owns DLOC=1250 hyper-dims (10 tiles of PT=125), all N rows.
    Host sorts rows by label so segment sums are contiguous-range sums.
  * mm1: z = projw_loc.T-tiles @ featT in fp32r (1 cyc/row), psum chunks
    [125, 1024].
  * sign: three engines in parallel convert z-psum into fp8 hv tiles:
      A-tiles (ACT):      Sign   -> hv in {-1, +1},  accum = S
      B-tiles (DVE/Pool): is_ge  -> hv in {0, 1},    accum = count = (S+len)/2
    (with accum_out, tensor_scalar's op1 is the reduce op, so one op can
    only produce {0,1}; the count->S shift folds into the host-side
    cwb = 2cw - len_k input for B tiles.)
  * mm2: fp8 DoubleRow matmuls (0.5 cyc/row, both d-tiles of a pair
    contracted per instr).  Stationary per pair: [125, 2, 12] fp8 packing
    hi/lo splits of u2 = 2*cw + S, with A-tiles in column block 0:6 and
    B-tiles in 6:12 -> psum rows 0:6 / 6:12.  A "ones" DoubleRow matmul
    gives the exact device-side column sums of the B stationaries.
    Host: P2 = (p0+p3) + 2*(p6+p9) - (cs6+cs9),
          logits = P2 / (2*||updated||).
  * Dummy matmuls keep the PE pstate ramped across the finalize window
    between mm1 and mm2; p_out is drained straight from PSUM by DMAs
    issued on the (otherwise idle) gpsimd queue.
"""

import os
import sys

sys.path.insert(0, "/opt/trn_rl_repo")
os.environ.setdefault("MYCRO_LOCAL_CACHE", "1")

import numpy as np

import concourse.bass as bass
import concourse.tile as tile
from concourse import bacc
from concourse import mybir
from concourse.bass import MemorySpace
from concourse.bass_utils import run_bass_kernel_spmd

# ---------------------------------------------------------------- constants
N = 16384          # rows
C = 128            # feat dim (contraction)
D = 10000          # hyper dim
K = 3              # classes
NCORES = 8
DLOC = D // NCORES          # 1250 per core
PT = 125                    # partitions per d-tile
NT = DLOC // PT             # 10 d-tiles per core
NPAIR = NT // 2             # 5 fp8 DoubleRow tile pairs
SCH = 1024                  # mm1 superchunk columns (2 psum banks)
NJ = N // SCH               # 16 superchunks
MCH = 512                   # mm1 matmul / mm2 output chunk columns
NC2 = N // MCH              # 32 mm2 chunks
MM_DT = mybir.dt.float32r   # encode-matmul dtype (1 cyc/row)
FP8 = mybir.dt.float8e4

# Only ACT and DVE can read PSUM (GPSIMD/Pool cannot), so the sign work
# has exactly two exit lanes: even tiles on ACT (Sign, +-1), odd tiles
# on DVE (is_ge, {0,1}).  With both lanes saturated the sign phase is
# the kernel's critical path (~2x the PE's mm1 time).
A_TILES = (0, 2, 4, 6, 8)   # ACT-signed tiles (+-1); odd tiles are {0,1}
N_DUMMY = 16                # PE keep-warm matmuls between mm1 and mm2

LAM = 0.5
EPS = 1e-12

LAST_RESULTS = None         # BassKernelResults of the most recent run (for test.py)


def _superchunk_segments(cuts):
    """Per superchunk j: list of (s0, s1, cls) relative to the superchunk,
    split at sorted-label class boundaries."""
    table = []
    for j in range(NJ):
        lo, hi = j * SCH, (j + 1) * SCH
        pts = [lo] + [b for b in cuts if lo < b < hi] + [hi]
        segs = []
        for a, b in zip(pts[:-1], pts[1:]):
            cls = 0 if a < cuts[0] else (1 if a < cuts[1] else 2)
            segs.append((a - lo, b - lo, cls))
        table.append(segs)
    return table


def _assign_engines():
    """Fixed two-lane assignment: A-tiles on ACT, B-tiles on DVE."""
    return [["A" if t in A_TILES else "D" for t in range(NT)] for j in range(NJ)]


def build_nc(cuts):
    """Build the single-core Bass program (same for all cores; only DRAM
    inputs differ per core).  cuts = [c0, c0+c1] sorted-label boundaries."""
    nc = bacc.Bacc()
    featT = nc.dram_tensor("featT", [C, N], MM_DT, kind="ExternalInput")
    projwT = nc.dram_tensor("projwT", [C, DLOC], MM_DT, kind="ExternalInput")
    cwb = nc.dram_tensor("cwb", [PT, NT * K], mybir.dt.float32, kind="ExternalInput")
    p_out = nc.dram_tensor("p_out", [4 * K, N], mybir.dt.float32, kind="ExternalOutput")
    cs_out = nc.dram_tensor("cs_out", [4 * K, 1], mybir.dt.float32, kind="ExternalOutput")
    s_out = nc.dram_tensor("s_out", [PT, NT * K], mybir.dt.float32, kind="ExternalOutput")

    seg_table = _superchunk_segments(cuts)
    ncols = sum(len(s) for s in seg_table)        # accum columns per tile
    col_cls = [cls for segs in seg_table for (_a, _b, cls) in segs]
    eng = _assign_engines()

    with tile.TileContext(nc) as tc:
        with (
            tc.tile_pool(name="singles", bufs=1) as singles,
            tc.tile_pool(name="feat", bufs=2) as featp,
            tc.tile_pool(name="u2f", bufs=2) as u2fp,
            tc.tile_pool(name="pstage", bufs=4) as pstp,
        ):
            # hv tiles first so their SBUF byte offsets stay 16B-aligned
            # (DoubleRow rhs requires 2B-aligned partition addresses)
            hv = [singles.tile([PT, 2, N], FP8, name=f"hv{p}") for p in range(NPAIR)]
            projw_sb = singles.tile([C, DLOC], MM_DT)
            # all DMA issues ride the gpsimd SWDGE queue: walrus reassigns
            # hwdge (SP) DMAs onto the ACT sequencer, which must stay free
            # for sign work
            nc.gpsimd.dma_start(out=projw_sb[:, :5 * PT], in_=projwT[:, :5 * PT])
            cwb_sb = singles.tile([PT, NT * K], mybir.dt.float32)
            s_sb = singles.tile([PT, NT * K], mybir.dt.float32)
            spart = singles.tile([PT, NT * ncols], mybir.dt.float32)
            # DoubleRow lhsT outer free step must be 16B-aligned -> pad the
            # per-plane stationary stride from 12 to 16 columns
            stat = [singles.tile([PT, 2, 16], FP8, name=f"st{p}") for p in range(NPAIR)]
            mones = singles.tile([PT, 2, 16], FP8)
            dums = singles.tile([C, MCH], mybir.dt.bfloat16)
            for p in range(NPAIR):
                nc.vector.memset(stat[p], 0.0)
            nc.vector.memset(mones, 1.0)
            nc.vector.memset(dums, 0.0)

            # ---- produce: z psum chunks -> fp8 hv tiles + segment sums ----
            with tc.tile_pool(name="mm1ps", bufs=4, space=MemorySpace.PSUM) as mm1ps:
                for j in range(NJ):
                    fj = featp.tile([C, SCH], MM_DT, tag="fj")
                    if j == 0:
                        # half-DMAs so mm1(0,0) starts after 256 KB
                        nc.gpsimd.dma_start(
                            out=fj[:, :MCH], in_=featT[:, :MCH])
                        nc.gpsimd.dma_start(
                            out=fj[:, MCH:], in_=featT[:, MCH:SCH])
                        nc.gpsimd.dma_start(
                            out=projw_sb[:, 5 * PT:], in_=projwT[:, 5 * PT:]
                        )
                    else:
                        nc.gpsimd.dma_start(
                            out=fj, in_=featT[:, j * SCH:(j + 1) * SCH])
                    if j == 2:
                        nc.gpsimd.dma_start(out=cwb_sb, in_=cwb[:, :])
                    for t in range(NT):
                        ps = mm1ps.tile([PT, SCH], mybir.dt.float32, tag="mm1")
                        for h in range(SCH // MCH):
                            nc.tensor.matmul(
                                ps[:, h * MCH:(h + 1) * MCH],
                                projw_sb[:, t * PT:(t + 1) * PT],
                                fj[:, h * MCH:(h + 1) * MCH],
                                start=True, stop=True,
                            )
                        col0 = sum(len(seg_table[jj]) for jj in range(j))
                        e = eng[j][t]
                        for si, (s0, s1, _cls) in enumerate(seg_table[j]):
                            hv_sl = hv[t // 2][:, t % 2, j * SCH + s0: j * SCH + s1]
                            acc = spart[:, t * ncols + col0 + si: t * ncols + col0 + si + 1]
                            if e == "A":
                                nc.scalar.activation(
                                    hv_sl, ps[:, s0:s1],
                                    mybir.ActivationFunctionType.Sign,
                                    accum_out=acc,
                                )
                            else:
                                # {0,1} in one op; accum = count of positives
                                # (op1 is the accum reduce op, not elementwise)
                                nc.vector.tensor_scalar(
                                    hv_sl, ps[:, s0:s1], 0.0, None,
                                    mybir.AluOpType.is_ge, mybir.AluOpType.add,
                                    accum_out=acc,
                                )

            # ---- per-tile finalize: S, u2/2 = m*s + cwb, fp8 hi/lo split --
            # (fp8e4 is IEEE e4m3, max 240; u2 reaches ~310, so the
            #  stationaries hold u2/2 and the host doubles the blocks)
            # Both exit lanes are still draining the last signs here, so
            # A-tile reduces ride ACT's Copy+accum path, B-tile reduces use
            # DVE's reduce, and the fp8 quantization runs on the idle Pool.
            red_scratch = singles.tile([PT, 8], mybir.dt.float32)
            for t in range(NT):
                is_a = t in A_TILES
                m_t = 0.5 if is_a else 1.0
                for k in range(K):
                    idxs = [i for i, cc in enumerate(col_cls) if cc == k]
                    a, b = idxs[0], idxs[-1] + 1
                    s_col = s_sb[:, t * K + k: t * K + k + 1]
                    cols = spart[:, t * ncols + a: t * ncols + b]
                    if is_a:
                        nc.scalar.activation(
                            red_scratch[:, :b - a], cols,
                            mybir.ActivationFunctionType.Copy,
                            accum_out=s_col,
                        )
                    else:
                        nc.vector.reduce_sum(
                            s_col, cols, axis=mybir.AxisListType.X,
                        )
                u2f = u2fp.tile([PT, K], mybir.dt.float32, tag="u2f")
                nc.vector.scalar_tensor_tensor(
                    u2f, s_sb[:, t * K:(t + 1) * K], m_t,
                    cwb_sb[:, t * K:(t + 1) * K],
                    mybir.AluOpType.mult, mybir.AluOpType.add,
                )
                blk = 0 if is_a else 2 * K   # column block in stationary
                st = stat[t // 2]
                hi = st[:, t % 2, blk: blk + K]
                lo = st[:, t % 2, blk + K: blk + 2 * K]
                nc.scalar.activation(
                    hi, u2f, mybir.ActivationFunctionType.Copy
                )
                nc.vector.scalar_tensor_tensor(
                    lo, u2f, 1.0, hi,
                    mybir.AluOpType.mult, mybir.AluOpType.subtract,
                )

            nc.gpsimd.dma_start(out=s_out[:, :], in_=s_sb)

            with tc.tile_pool(name="pps", bufs=4, space=MemorySpace.PSUM) as pps:
                # ---- PE keep-warm bridge over the finalize window ---------
                for i in range(N_DUMMY):
                    dpp = pps.tile([128, MCH], mybir.dt.float32, tag="pp")
                    nc.tensor.matmul(
                        dpp[:PT, :], dums[:, 0:PT], dums,
                        start=True, stop=True,
                    )

                # ---- stationary column sums (exact {0,1} fixup) -----------
                ppc = pps.tile([128, MCH], mybir.dt.float32, tag="pp")
                for p in range(NPAIR):
                    nc.tensor.matmul(
                        ppc[:4 * K, 0:1], stat[p][:, :, 0:4 * K],
                        mones[:, :, 0:1],
                        start=(p == 0), stop=(p == NPAIR - 1),
                        perf_mode=mybir.MatmulPerfMode.DoubleRow,
                    )
                csst = pstp.tile([4 * K, 1], mybir.dt.float32, tag="csst")
                nc.vector.tensor_copy(csst, ppc[:4 * K, 0:1])
                nc.gpsimd.dma_start(out=cs_out[:, :], in_=csst)

                # ---- mm2: P2 partials via fp8 DoubleRow -------------------
                # drain copies alternate DVE/ACT; two chunks share one
                # staging tile; DMA issues alternate SP / gpsimd queues
                GRP = 2
                pst = None
                for c in range(NC2):
                    pp = pps.tile([128, MCH], mybir.dt.float32, tag="pp")
                    for p in range(NPAIR):
                        nc.tensor.matmul(
                            pp[:4 * K, :], stat[p][:, :, 0:4 * K],
                            hv[p][:, :, c * MCH:(c + 1) * MCH],
                            start=(p == 0), stop=(p == NPAIR - 1),
                            perf_mode=mybir.MatmulPerfMode.DoubleRow,
                        )
                    if c % GRP == 0:
                        pst = pstp.tile([4 * K, GRP * MCH], mybir.dt.float32, tag="pst")
                    half = pst[:, (c % GRP) * MCH:(c % GRP + 1) * MCH]
                    # only ACT/DVE can read PSUM
                    if c % 2 == 0:
                        nc.vector.tensor_copy(half, pp[:4 * K, :])
                    else:
                        nc.scalar.activation(
                            half, pp[:4 * K, :], mybir.ActivationFunctionType.Copy
                        )
                    if c % GRP == GRP - 1:
                        nc.gpsimd.dma_start(
                            out=p_out[:, (c - GRP + 1) * MCH:(c + 1) * MCH],
                            in_=pst,
                        )
    nc.compile()
    return nc


def _prep_inputs(feat_s, proj_w, classify_weights, counts):
    featT = np.ascontiguousarray(feat_s.T).astype(np.float32)  # [128, N]
    in_maps = []
    for core in range(NCORES):
        sl = slice(core * DLOC, (core + 1) * DLOC)
        projwT = np.ascontiguousarray(proj_w[sl].T).astype(np.float32)  # [128, DLOC]
        # stationaries hold u2/2 = cw + S/2; for {0,1} tiles S = 2c - len
        cw2 = classify_weights[:, sl].astype(np.float32).T              # [DLOC, 3]
        for t in range(NT):
            if t not in A_TILES:
                cw2[t * PT:(t + 1) * PT, :] -= 0.5 * counts[None, :].astype(np.float32)
        cwb = np.ascontiguousarray(
            cw2.reshape(NT, PT, K).transpose(1, 0, 2).reshape(PT, NT * K)
        )
        in_maps.append({"featT": featT, "projwT": projwT, "cwb": cwb})
    return in_maps


def kernel(feat, proj_w, classify_weights, labels, _trace=False):
    global LAST_RESULTS
    feat = np.asarray(feat, dtype=np.float32)
    proj_w = np.asarray(proj_w, dtype=np.float32)
    classify_weights = np.asarray(classify_weights, dtype=np.float32)
    labels = np.asarray(labels).astype(np.int64)

    perm = np.argsort(labels, kind="stable")
    feat_s = feat[perm]
    counts = np.bincount(labels, minlength=K)
    cuts = [int(counts[0]), int(counts[0] + counts[1])]

    nc = build_nc(cuts)
    in_maps = _prep_inputs(feat_s, proj_w, classify_weights, counts)
    res = run_bass_kernel_spmd(nc, in_maps, list(range(NCORES)), trace=_trace)
    LAST_RESULTS = res

    S = np.zeros((K, D), np.float32)
    P2 = np.zeros((K, N), np.float64)
    for core in range(NCORES):
        s_raw = np.asarray(res.results[core]["s_out"])        # [PT, NT*K]
        s_full = s_raw.reshape(PT, NT, K).transpose(1, 0, 2)  # [NT, PT, K]
        for t in range(NT):
            if t in A_TILES:
                st = s_full[t]
            else:
                st = 2.0 * s_full[t] - counts[None, :].astype(np.float32)
            S[:, core * DLOC + t * PT: core * DLOC + (t + 1) * PT] = st.T
        p = np.asarray(res.results[core]["p_out"]).astype(np.float64)   # [12, N]
        cs = np.asarray(res.results[core]["cs_out"]).astype(np.float64)  # [12, 1]
        P2 += 2.0 * ((p[0:3] + p[3:6]) + 2.0 * (p[6:9] + p[9:12])
                     - (cs[6:9] + cs[9:12]))

    updated = classify_weights + np.float32(LAM) * S          # [K, D] f32
    norms = np.linalg.norm(updated, axis=1)
    scale = 0.5 / np.maximum(norms, EPS)
    logits_sorted = (P2 * scale[:, None]).T.astype(np.float32)  # [N, K]
    out = np.empty((N, K), np.float32)
    out[perm] = logits_sorted
    return out
